# revision 28
# baseline (speedup 1.0000x reference)
"""GQA attention block (B=2,S=2048,D=4096,H=32,KVH=8,HD=128) on 8 trn2 cores.

Sharding: core c -> batch b=c//4, head-group g=c%4 (8 q heads, 2 kv heads per
core).  Each core computes QKV projections + RoPE + causal attention + its
slice of the output projection; the host sums the 4 partial outputs per batch.
"""

import numpy as np
import ml_dtypes

import concourse.bass as bass
import concourse.tile as tile
import concourse.mybir as mybir
from concourse import bacc
from concourse.bass_utils import run_bass_kernel_spmd
from concourse.masks import make_identity

F32 = mybir.dt.float32
F32R = mybir.dt.float32r
BF16 = mybir.dt.bfloat16
AX = mybir.AxisListType
AF = mybir.ActivationFunctionType

B, S, D = 2, 2048, 4096
H, KVH, HD = 32, 8, 128
N_REP = H // KVH
N_CORES = 8
NH = 8            # q heads per core
NKV = 2           # kv heads per core
TP = 256          # qkv token-pass width
NTP = S // TP
DCH = D // 128    # contraction chunks
NQT = S // 128    # q tiles
NKT = S // 128    # k tiles
NCT = NH + 2 * NKV  # projection col-tiles: 8 q, 2 k, 2 v


def _mm_chunks(start, end):
    """Split [start, end) into matmul col ranges that never cross a 512
    boundary (PSUM bank limit for fp32 outputs)."""
    out = []
    c = start
    while c < end:
        w = min(end, (c // 512 + 1) * 512) - c
        out.append((c, w))
        c += w
    return out


def _build(causal: bool, repeat: int = 1):
    nc = bacc.Bacc(None, target_bir_lowering=False, debug=False)

    TPW = 1024                # qkv token-pass width
    NTPW = S // TPW

    xT = nc.dram_tensor("xT", [D, S], BF16, kind="ExternalInput")
    # weights staged as per-col-tile slabs: slab[c*128+p, d*128+j] = w[d*128+p, c*128+j]
    # -> one contiguous-line DMA loads all 32 contraction tiles of col-tile c
    wq = nc.dram_tensor("wq", [NH * 128, DCH * 128], BF16, kind="ExternalInput")
    wk = nc.dram_tensor("wk", [NKV * 128, DCH * 128], BF16, kind="ExternalInput")
    wv = nc.dram_tensor("wv", [NKV * 128, DCH * 128], BF16, kind="ExternalInput")
    wo = nc.dram_tensor("wo", [NH * HD, D], BF16, kind="ExternalInput")
    cos_rep = nc.dram_tensor("cos_rep", [128, S], F32, kind="ExternalInput")
    sin_rep = nc.dram_tensor("sin_rep", [128, S], F32, kind="ExternalInput")
    if causal:
        mask_t_in = nc.dram_tensor("mask_diag_t", [128, S], BF16, kind="ExternalInput")
    else:
        mask_t_in = nc.dram_tensor("mask_full_t", [S, S], F32, kind="ExternalInput")
    out = nc.dram_tensor("out", [S, D], F32, kind="ExternalOutput")

    l_dram = nc.dram_tensor("l_scratch", [NH, S], F32)

    oT_dram = nc.dram_tensor("oT_scratch", [NH, 128, S], BF16)
    
    with tile.TileContext(nc) as tc:
        with (
            tc.tile_pool(name="const", bufs=1) as constp,
            tc.tile_pool(name="small", bufs=10) as small,
        ):
            ident_f = constp.tile([128, 128], F32, tag="ident_f")
            make_identity(nc, ident_f[:])
            ident_r = constp.tile([128, 128], F32R, tag="ident_r")
            nc.vector.tensor_copy(ident_r[:], ident_f[:])
            ones_sb = constp.tile([128, 1], BF16, tag="ones")
            nc.vector.memset(ones_sb[:], 1.0)

            for _rep in range(repeat):
                with (
                    tc.tile_pool(name="acts", bufs=1) as acts,
                ):
                    qT = [acts.tile([128, S], BF16, tag=f"qT{h}", name=f"qT{h}")
                          for h in range(NH)]
                    kT = [acts.tile([128, S], BF16, tag=f"kT{k}", name=f"kT{k}")
                          for k in range(NKV)]
                    v_sb = [acts.tile([128, S], BF16, tag=f"v{k}", name=f"v{k}")
                            for k in range(NKV)]

                    # ---------- Phase 1: QKV projection + RoPE ----------
                    with (
                        tc.tile_pool(name="xq", bufs=DCH + 2) as xqp,
                        tc.tile_pool(name="wslab", bufs=3) as wslabp,
                        tc.tile_pool(name="rope", bufs=2) as ropep,
                        tc.tile_pool(name="trig", bufs=1) as trigp,
                        tc.tile_pool(name="vtmp", bufs=2) as vtmpp,
                        tc.tile_pool(name="ps_qkv", bufs=2, space="PSUM") as psq,
                        tc.tile_pool(name="ps_v", bufs=2, space="PSUM") as psv,
                    ):
                        def rope_evict(ps, dest, cos_t, sin_t):
                            qc = ropep.tile([128, TPW], F32, tag="qc", name="qc")
                            qs = ropep.tile([128, TPW], F32, tag="qs", name="qs")
                            qsw = ropep.tile([128, TPW], F32, tag="qsw", name="qsw")
                            nc.vector.tensor_mul(qc[:], ps[:], cos_t[:])
                            nc.vector.tensor_mul(qs[:], ps[:], sin_t[:])
                            nc.gpsimd.dma_start(qsw[0:64, :], qs[64:128, :])
                            nc.gpsimd.dma_start(qsw[64:128, :], qs[0:64, :])
                            nc.vector.tensor_sub(dest[0:64, :], qc[0:64, :], qsw[0:64, :])
                            nc.vector.tensor_add(dest[64:128, :], qc[64:128, :], qsw[64:128, :])

                        cos_full = trigp.tile([128, S], F32, tag="cos", name="cos_full")
                        sin_full = trigp.tile([128, S], F32, tag="sin", name="sin_full")
                        slab0 = wslabp.tile([128, DCH * 128], BF16,
                                            tag="wslab", name="slab")
                        nc.sync.dma_start(slab0[:], wk.ap()[0:128, :])
                        nc.sync.dma_start(cos_full[:], cos_rep.ap())
                        nc.sync.dma_start(sin_full[:], sin_rep.ap())
                        for tp in range(NTPW):
                            t0 = tp * TPW
                            cos_t = cos_full[:, t0:t0 + TPW]
                            sin_t = sin_full[:, t0:t0 + TPW]
                            xt = []
                            for d in range(DCH):
                                xd = xqp.tile([128, TPW], BF16, tag="x", name="xd")
                                nc.scalar.dma_start(
                                    xd[:], xT.ap()[d * 128:(d + 1) * 128, t0:t0 + TPW])
                                xt.append(xd)

                            for ct in list(range(NH, NCT)) + list(range(NH)):
                                if ct < NH:
                                    wsrc, col, is_q, is_rope = wq, ct, True, True
                                elif ct < NH + NKV:
                                    k = ct - NH
                                    wsrc, col, is_q, is_rope = wk, k, False, True
                                else:
                                    k = ct - NH - NKV
                                    wsrc, col, is_q, is_rope = wv, k, False, False

                                if tp == 0 and ct == NH:
                                    slab = slab0
                                else:
                                    slab = wslabp.tile([128, DCH * 128], BF16,
                                                       tag="wslab", name="slab")
                                    nc.sync.dma_start(
                                        slab[:],
                                        wsrc.ap()[col * 128:(col + 1) * 128, :])
                                ps = psq.tile([128, TPW], F32, tag="ps", name="ps")
                                for d in range(DCH):
                                    for off in range(0, TPW, 512):
                                        nc.tensor.matmul(
                                            ps[:, off:off + 512],
                                            slab[:, d * 128:(d + 1) * 128],
                                            xt[d][:, off:off + 512],
                                            start=(d == 0), stop=(d == DCH - 1))

                                if is_q:
                                    rope_evict(ps, qT[ct][:, t0:t0 + TPW], cos_t, sin_t)
                                elif is_rope:
                                    rope_evict(ps, kT[k][:, t0:t0 + TPW], cos_t, sin_t)
                                else:
                                    vt = vtmpp.tile([128, TPW], F32R, tag="vt", name="vt")
                                    nc.scalar.copy(vt[:], ps[:])
                                    for kk in range(TPW // 128):
                                        tt = (t0 + kk * 128) // 128
                                        pv = psv.tile([128, 128], F32R, tag="pv", name="pv")
                                        nc.tensor.transpose(
                                            pv[:], vt[:, kk * 128:(kk + 1) * 128], ident_r[:])
                                        nc.scalar.copy(
                                            v_sb[k][:, tt * 128:(tt + 1) * 128], pv[:])

                    # ---------- Phases 2+3 share resident wo ----------
                    with (
                        tc.tile_pool(name="wof", bufs=1) as wof,
                        tc.tile_pool(name="otsb", bufs=2) as otsbp,
                        tc.tile_pool(name="otp", bufs=6) as otp,
                    ):
                        wo_sb = [wof.tile([128, D], BF16, tag=f"wo{h2}",
                                          name=f"wo{h2}") for h2 in range(NH)]
                        oT_keep = {}

                        # ---------- Phase 2: attention ----------
                        # Single s^T pass, bias-free exp (max logit ~11 << 88
                        # so no max subtraction needed), row-sums l via
                        # ones-matmul accumulated alongside o^T, 1/l applied
                        # once per [128,1024] o^T block.  PSUM evictions go to
                        # the idle Pool engine; normalize-muls are deferred one
                        # head so the DVE in-order queue never blocks on the
                        # l broadcast DMA.
                        with (
                            tc.tile_pool(name="maskp",
                                         bufs=1 if causal else 6) as maskp,
                            tc.tile_pool(name="ptp", bufs=3) as ptp,
                            tc.tile_pool(name="otf", bufs=2) as otfp,
                            tc.tile_pool(name="statp", bufs=4) as statp,
                            tc.tile_pool(name="lrepp", bufs=2) as lrepp,
                            tc.tile_pool(name="ps_s", bufs=2, space="PSUM") as pss,
                            tc.tile_pool(name="ps_ot", bufs=1, space="PSUM") as psot,
                            tc.tile_pool(name="ps_l", bufs=1, space="PSUM") as psl,
                        ):
                            if causal:
                                mask_t_sb = maskp.tile([128, S], BF16, tag="mask_t")
                                nc.sync.dma_start(mask_t_sb[:], mask_t_in.ap())

                            pending = []   # deferred (dest, otf, lrep) muls
                            oT_store = []  # deferred oT_dram stores

                            oT_pre = {}

                            def flush_head():
                                while pending:
                                    dest, otf_, lrep_ = pending.pop(0)
                                    nc.vector.tensor_mul(dest, otf_[:], lrep_[:])
                                while oT_store:
                                    hh, sb = oT_store.pop(0)
                                    nc.sync.dma_start(oT_dram.ap()[hh], sb[:])
                                    # prefetch this head's tg=0 slice for the
                                    # output projection on the Act DGE queue
                                    o = otp.tile([128, 1024], BF16, tag="o",
                                                 name="o")
                                    nc.scalar.dma_start(
                                        o[:], oT_dram.ap()[hh, :, 0:1024])
                                    oT_pre[hh] = o

                            # flattened (h, qh, ki) step list, software-pipelined
                            # with one-step s-matmul lookahead so the exp
                            # latency hides behind the next s instead of
                            # stalling the in-order PE queue at PV.
                            steps = []
                            for h in range(NH):
                                for qh in (1, 0):  # qh=1 first: hides mask load
                                    qlo, qhi = qh * 1024, (qh + 1) * 1024
                                    for ki in range(NKT):
                                        q0 = max(ki * 128 if causal else qlo, qlo)
                                        if q0 >= qhi:
                                            continue
                                        steps.append((h, qh, ki, q0, qlo, qhi))

                            sps = {}  # step idx -> (sp tile, pt tile)

                            def issue_s(i):
                                h, qh, ki, q0, qlo, qhi = steps[i]
                                cw = qhi - q0
                                sp = pss.tile([128, 1024], F32, tag="s", name="sp")
                                for off, w in _mm_chunks(0, cw):
                                    nc.tensor.matmul(
                                        sp[:, off:off + w],
                                        kT[h // N_REP][:, ki * 128:(ki + 1) * 128],
                                        qT[h][:, q0 + off:q0 + off + w],
                                        start=True, stop=True)
                                if not causal:
                                    mt = maskp.tile([128, 1024], F32,
                                                    tag="mask_t", name="mt")
                                    nc.sync.dma_start(
                                        mt[:, :cw],
                                        mask_t_in.ap()[ki * 128:(ki + 1) * 128,
                                                       q0:qhi])
                                    nc.vector.tensor_add(
                                        sp[:, :cw], sp[:, :cw], mt[:, :cw])
                                sps[i] = sp

                            cur = {}  # live per-qh state: ot, l_ps, otf, lrcp
                            oT_sb = None
                            for i, (h, qh, ki, q0, qlo, qhi) in enumerate(steps):
                                first_of_head = i == 0 or steps[i - 1][0] != h
                                first_of_qh = first_of_head or steps[i - 1][1] != qh
                                if first_of_head:
                                    flush_head()
                                    nc.scalar.dma_start(
                                        wo_sb[h][:],
                                        wo.ap()[h * HD:(h + 1) * HD, :])
                                    oT_sb = otsbp.tile([128, S], BF16, tag="oT_ev",
                                                       name=f"oT_sb{h}")
                                    oT_keep[h] = oT_sb
                                if first_of_qh:
                                    cur['ot'] = psot.tile([128, 1024], F32,
                                                          tag="ot", name=f"ot{qh}")
                                    cur['l'] = psl.tile([1, 1024], F32, tag="l",
                                                        name="l_ps")
                                    cur['otf'] = otfp.tile([128, 1024], F32,
                                                           tag="otf", name="otf")
                                    cur['lrcp'] = statp.tile([1, 1024], F32,
                                                             tag="lrcp", name="lrcp")
                                if i == 0:
                                    issue_s(0)
                                if i + 1 < len(steps):
                                    issue_s(i + 1)

                                sp = sps.pop(i)
                                cw = qhi - q0
                                ot, l_ps = cur['ot'], cur['l']
                                otf, lrcp = cur['otf'], cur['lrcp']
                                pt = ptp.tile([128, 1024], BF16, tag="pt",
                                              name="pt")
                                nc.scalar.activation(pt[:, :cw], sp[:, :cw], AF.Exp)
                                if causal and q0 == ki * 128:
                                    nc.vector.tensor_mul(
                                        pt[:, 0:128], pt[:, 0:128],
                                        mask_t_sb[:, ki * 128:(ki + 1) * 128])
                                chunks = _mm_chunks(q0, qhi)
                                for c, w in chunks:
                                    co = c - qlo
                                    nc.tensor.matmul(
                                        ot[:, co:co + w],
                                        v_sb[h // N_REP][:, ki * 128:(ki + 1) * 128],
                                        pt[:, c - q0:c - q0 + w],
                                        start=(ki == 0),
                                        stop=(ki == ((c + w - 1) // 128
                                                     if causal else NKT - 1)),
                                        skip_group_check=True)
                                for c, w in chunks:
                                    co = c - qlo
                                    last_ki = ((c + w - 1) // 128 if causal
                                               else NKT - 1)
                                    nc.tensor.matmul(
                                        l_ps[0:1, co:co + w],
                                        ones_sb[:, 0:1],
                                        pt[:, c - q0:c - q0 + w],
                                        start=(ki == 0), stop=(ki == last_ki),
                                        skip_group_check=True)
                                    # evict each 512-half as soon as its
                                    # accumulation closes (hides the qh-boundary
                                    # psum eviction)
                                    if ki == last_ki:
                                        hb = co // 512 * 512
                                        nc.vector.tensor_copy(
                                            otf[:, hb:hb + 512],
                                            ot[:, hb:hb + 512])
                                        nc.vector.reciprocal(
                                            lrcp[0:1, hb:hb + 512],
                                            l_ps[0:1, hb:hb + 512])

                                last_of_qh = (i + 1 == len(steps)
                                              or steps[i + 1][1] != qh
                                              or steps[i + 1][0] != h)
                                if last_of_qh:
                                    nc.sync.dma_start(
                                        l_dram.ap()[h:h + 1, qlo:qhi],
                                        lrcp[0:1, :])
                                    lrep = lrepp.tile([128, 1024], F32,
                                                      tag="lrep", name="lrep")
                                    nc.sync.dma_start(
                                        lrep[:],
                                        l_dram.ap()[h:h + 1, qlo:qhi]
                                        .to_broadcast((128, 1024)))
                                    pending.append(
                                        (oT_sb[:, qlo:qhi], otf, lrep))
                                last_of_head = (i + 1 == len(steps)
                                                or steps[i + 1][0] != h)
                                if last_of_head and h < NH - 2:
                                    oT_store.append((h, oT_sb))
                            flush_head()

                        # ---------- Phase 3: output projection ----------
                        with (
                            tc.tile_pool(name="outp", bufs=2) as outp,
                            tc.tile_pool(name="ps_out", bufs=2,
                                         space="PSUM") as psout,
                        ):
                            for tg in range(2):
                                oT_tiles = []
                                for h2 in range(NH):
                                    if h2 >= NH - 2:  # last 2 heads never left SBUF
                                        oT_tiles.append((oT_keep[h2], tg * 1024))
                                        continue
                                    if tg == 0:      # prefetched during attention
                                        oT_tiles.append((oT_pre[h2], 0))
                                        continue
                                    o = otp.tile([128, 1024], BF16, tag="o",
                                                 name="o")
                                    nc.scalar.dma_start(
                                        o[:],
                                        oT_dram.ap()[h2, :, tg * 1024:(tg + 1) * 1024])
                                    oT_tiles.append((o, 0))
                                for tl in range(8):
                                    tt = tg * 8 + tl
                                    for half in range(2):
                                        po = psout.tile([128, 2048], F32,
                                                        tag="po", name="po")
                                        for h2 in range(NH):
                                            otile, ob = oT_tiles[h2]
                                            lhsT = otile[:, ob + tl * 128:
                                                         ob + (tl + 1) * 128]
                                            for dj in range(4):
                                                nc.tensor.matmul(
                                                    po[:, dj * 512:(dj + 1) * 512],
                                                    lhsT,
                                                    wo_sb[h2][:, half * 2048 + dj * 512:
                                                              half * 2048 + (dj + 1) * 512],
                                                    start=(h2 == 0),
                                                    stop=(h2 == NH - 1),
                                                    skip_group_check=True)
                                        osb = outp.tile([128, 2048], F32,
                                                        tag="osb", name="osb")
                                        if tg == 1 and tl == 7:
                                            # chunked drain for the tail, split
                                            # across Act and DVE
                                            for dj in range(4):
                                                dl = dj * 512
                                                eng = (nc.scalar.copy if dj % 2
                                                       else nc.vector.tensor_copy)
                                                eng(osb[:, dl:dl + 512],
                                                    po[:, dl:dl + 512])
                                                nc.sync.dma_start(
                                                    out.ap()[tt * 128:(tt + 1) * 128,
                                                             half * 2048 + dl:
                                                             half * 2048 + dl + 512],
                                                    osb[:, dl:dl + 512])
                                        else:
                                            nc.scalar.copy(osb[:], po[:])
                                            nc.sync.dma_start(
                                                out.ap()[tt * 128:(tt + 1) * 128,
                                                         half * 2048:(half + 1) * 2048],
                                                osb[:])

    nc.compile()
    return nc


def _is_causal(mask: np.ndarray) -> bool:
    if mask.shape != (S, S):
        return False
    neg = mask[0, 1]
    if not (neg <= -1e8):
        return False
    expect = np.triu(np.full((S, S), neg, dtype=np.float32), 1)
    return np.array_equal(mask, expect)


_PROG = {}


def _get_prog(causal: bool, repeat: int = 1):
    key = (causal, repeat)
    if key not in _PROG:
        _PROG[key] = _build(causal, repeat)
    return _PROG[key]


def _stage(x, cos, sin, mask, wq, wk, wv, wo, causal):
    perm = np.concatenate([np.arange(0, HD, 2), np.arange(1, HD, 2)])
    # fold the 1/sqrt(HD) attention scale into wq (RoPE is linear in q)
    wq_p = (wq * np.float32(1.0 / np.sqrt(HD))).reshape(D, H, HD)[:, :, perm]
    wk_p = wk.reshape(D, KVH, HD)[:, :, perm]
    wv_r = wv.reshape(D, KVH, HD)

    cos_rep = np.ascontiguousarray(
        np.concatenate([cos.T, cos.T], axis=0), dtype=np.float32)
    sin_rep = np.ascontiguousarray(
        np.concatenate([sin.T, sin.T], axis=0), dtype=np.float32)

    if causal:
        mask_diag_t = np.empty((128, S), dtype=np.float32)
        for qi in range(NQT):
            blk = mask[qi * 128:(qi + 1) * 128, qi * 128:(qi + 1) * 128]
            mask_diag_t[:, qi * 128:(qi + 1) * 128] = (blk.T == 0.0)
        mask_diag_t = mask_diag_t.astype(ml_dtypes.bfloat16)
    else:
        mask_full_t = np.ascontiguousarray(mask.T, dtype=np.float32)

    xT = [
        np.ascontiguousarray(x[b].T).astype(ml_dtypes.bfloat16) for b in range(B)
    ]

    def tile_layout(w, ncols):
        # [D, ncols*128] -> [ncols*128, DCH*128] slabs:
        # slab[c*128+p, d*128+j] = w[d*128+p, c*128+j]
        return np.ascontiguousarray(
            w.reshape(DCH, 128, ncols, 128).transpose(2, 1, 0, 3)
            .reshape(ncols * 128, DCH * 128))

    in_maps = []
    for c in range(N_CORES):
        b, g = c // 4, c % 4
        m = {
            "xT": xT[b],
            "wq": tile_layout(
                wq_p[:, 8 * g:8 * g + 8].reshape(D, NH * HD), NH
            ).astype(ml_dtypes.bfloat16),
            "wk": tile_layout(
                wk_p[:, 2 * g:2 * g + 2].reshape(D, NKV * HD), NKV
            ).astype(ml_dtypes.bfloat16),
            "wv": tile_layout(
                wv_r[:, 2 * g:2 * g + 2].reshape(D, NKV * HD), NKV
            ).astype(ml_dtypes.bfloat16),
            "wo": np.ascontiguousarray(
                wo[1024 * g:1024 * (g + 1), :]).astype(ml_dtypes.bfloat16),
            "cos_rep": cos_rep,
            "sin_rep": sin_rep,
        }
        if causal:
            m["mask_diag_t"] = mask_diag_t
        else:
            m["mask_full_t"] = mask_full_t
        in_maps.append(m)
    return in_maps


def _run(inputs, trace=False):
    x = np.asarray(inputs["x"], dtype=np.float32)
    cos = np.asarray(inputs["cos"], dtype=np.float32)
    sin = np.asarray(inputs["sin"], dtype=np.float32)
    mask = np.asarray(inputs["mask"], dtype=np.float32)
    wq = np.asarray(inputs["wq"], dtype=np.float32)
    wk = np.asarray(inputs["wk"], dtype=np.float32)
    wv = np.asarray(inputs["wv"], dtype=np.float32)
    wo = np.asarray(inputs["wo"], dtype=np.float32)

    causal = _is_causal(mask)
    nc = _get_prog(causal)
    in_maps = _stage(x, cos, sin, mask, wq, wk, wv, wo, causal)
    res = run_bass_kernel_spmd(nc, in_maps, list(range(N_CORES)), trace=trace)

    out = np.empty((B, S, D), dtype=np.float32)
    for b in range(B):
        acc = res.results[4 * b]["out"].astype(np.float32).copy()
        for g in range(1, 4):
            acc += res.results[4 * b + g]["out"]
        out[b] = acc
    return out, res


def kernel(**inputs) -> np.ndarray:
    out, _ = _run(inputs, trace=False)
    return out

